# revision 59
# baseline (speedup 1.0000x reference)
"""Trainium2 Bass kernel for nn_MixedLinear (DARTS-style mixed-precision supernet linear).

Reference math (16-term arch-weighted mixture) reduces algebraically to:

  x_mix = C * round(x)                      C = sum(arch_weights)
  w_mix[o,i] = G0(R,Cc)*s0*clip(round(w/s0),-8,7) + G1(R,Cc)*s1*round(w/s1)
  out = x_mix @ w_mix^T + beta(R) * bias
      = round(x) @ W_eff^T + b_mix,   W_eff = C * w_mix

with region grid R = (o >= 3072), Cc = (i >= 768); see the fallback for the
unreduced form.

v4 design (vs the fp32r data-parallel v1 at ~170us):
  - Hybrid sharding: tokens 4-way x output-rows 2-way, with the o-rows
    assigned in INTERLEAVED 512-row blocks (core half jo takes blocks
    jo, jo+2, jo+4, jo+6). Both halves then see the same R-pattern
    (R=1 only in their last block), so a single SPMD program with the
    (R,Cc) dequant scales baked as immediates serves all 8 cores.
    Halves the per-core W-dequant elementwise work and HBM vs pure DP.
  - bf16 stationary (W_eff) and moving (round(x), exact in bf16): the PE
    streams 1 row/cycle either way but LDWEIGHTS halves, and W_eff
    rounding to bf16 costs only ~1e-3 relative output error.
    (fp8+DoubleRow was tried: DR matmuls measure 216ns vs bf16 259ns per
    512-row instr, but the A+B residual split doubles the FLOPs and
    fp8 elementwise on DVE/Pool is 4-8x slower than fp32 -- net loss.)
  - Output written bf16 (halves output HBM); host upcasts.
  - Elementwise spread across Act/DVE/Pool (Pool gets float-immediate
    tensor_scalars only); PSUM drains alternate Act/DVE and apply the
    beta-folded host-prepared bias in the same op.
  - First span's chains are balanced for latency so the PE starts ~10us
    in; later spans' chains lean on Pool (which idles during matmuls).

Rounding on device uses the magic-number trick: fp32 (v + 1.5*2^23) -
1.5*2^23 == round-half-even(v), matching jnp.round exactly.
"""

import numpy as np

import concourse.mybir as mybir
from concourse import bacc, bass_utils
from concourse.tile import TileContext

N_CORES = 8
NJT, NJO = 4, 2          # token shards x output shards
B, S, I_DIM, O_DIM = 4, 2048, 1024, 4096
T_TOT = B * S
T_CORE = T_TOT // NJT    # 2048 tokens per core
O_CORE = O_DIM // NJO    # 2048 output rows per core
NI = I_DIM // 128        # 8 contraction tiles
NSP = 4                  # o-spans (interleaved 512-blocks) per core
O_SPAN = O_CORE // NSP   # 512
NOT_ = O_SPAN // 128     # 4 o-tiles per span
TCH = 512                # matmul moving free dim
NTC = T_CORE // TCH      # 4 t-chunks
MAGIC = 12582912.0       # 1.5 * 2**23
F32 = mybir.dt.float32
BF16 = mybir.dt.bfloat16
AL = mybir.AluOpType
AF = mybir.ActivationFunctionType

_cache: dict = {}
_last_res = None


def _build(inv_s0, inv_s1, q0, q1):
    """Build + compile the per-core kernel. q0/q1 are 2x2 (R, Cc) grids,
    baked as immediates; span sp uses R = (sp == NSP-1) on every core."""
    nc = bacc.Bacc("TRN2", target_bir_lowering=False)
    x_t = nc.dram_tensor("x_t", [I_DIM, T_CORE], F32, kind="ExternalInput")
    w_t = nc.dram_tensor("w_t", [I_DIM, O_CORE], F32, kind="ExternalInput")
    b_pt = nc.dram_tensor("b_pt", [128, O_CORE // 128], F32, kind="ExternalInput")
    out_t = nc.dram_tensor("out_t", [O_CORE, T_CORE], BF16, kind="ExternalOutput")

    with TileContext(nc, pool_alloc_mode="queue") as tc:
        with (
            tc.tile_pool(name="pper", bufs=1) as pper,
            tc.tile_pool(name="pxs", bufs=3) as pxs,
            tc.tile_pool(name="pws", bufs=4) as pws,
            tc.tile_pool(name="pwb", bufs=2) as pwb,
            tc.tile_pool(name="pt", bufs=3) as pt,
            tc.tile_pool(name="pob", bufs=5) as pob,
            tc.tile_pool(name="psum", bufs=8, space="PSUM") as psum,
        ):
            b_t = pper.tile([128, O_CORE // 128], F32, tag="bt")
            nc.sync.dma_start(out=b_t, in_=b_pt[:, :])

            # xq = round(x^T) in bf16 (exact: ints in [-7,7]). Layout [p, i*T+t].
            xq_all = pper.tile([128, NI * T_CORE], BF16, tag="xq")
            x_r = x_t[:, :].rearrange("(i p) t -> p i t", p=128)
            w_r = w_t[:, :].rearrange("(i p) o -> p i o", p=128)

            def load_x_chunk(tcn, nsplit=1):
                """Load one t-chunk for all 8 k-tiles with nsplit batched 3D
                DMAs (one SP issue covers NI/nsplit tiles), then round."""
                step = NI // nsplit
                xb = pxs.tile([128, NI * TCH], F32, tag="xb")
                for s in range(nsplit):
                    nc.sync.dma_start(
                        out=xb[:, s * step * TCH : (s + 1) * step * TCH].rearrange(
                            "p (i t) -> p i t", i=step
                        ),
                        in_=x_r[:, s * step : (s + 1) * step, TCH * tcn : TCH * (tcn + 1)],
                    )
                for i in range(NI):
                    o0 = i * T_CORE + TCH * tcn
                    # dtype-converting ops are fast only on DVE/Act (Pool's
                    # software path is ~12x slower for bf16 writes / max-min)
                    nc.vector.tensor_scalar(
                        xq_all[:, o0 : o0 + TCH],
                        xb[:, i * TCH : (i + 1) * TCH],
                        MAGIC, MAGIC, AL.add, AL.subtract,
                    )
                    yield

            # W_eff stationary bf16 per span, layout [p, i*O_SPAN + o]
            we_tiles = [
                pper.tile([128, NI * O_SPAN], BF16, tag=f"we{sp}", name=f"we{sp}")
                for sp in range(NSP)
            ]

            def quant_chain(sp, i, c_lo=0, c_hi=O_SPAN, wr=None):
                """Dequant chain for (sp, i), span-columns [c_lo, c_hi).
                max/min never goes to Pool (12x slow there); Pool gets only
                add/mult; Act gets the affine rounds; DVE clip + combine."""
                R = 1 if sp == NSP - 1 else 0
                Cc = 1 if i * 128 >= 768 else 0
                w_ = c_hi - c_lo
                if wr is None:
                    wr = pws.tile([128, w_], F32, tag="wr")
                    nc.sync.dma_start(
                        out=wr,
                        in_=w_t[
                            128 * i : 128 * (i + 1),
                            sp * O_SPAN + c_lo : sp * O_SPAN + c_hi,
                        ],
                    )
                r0 = pt.tile([128, w_], F32, tag="r0")
                nc.scalar.activation(r0, wr, AF.Copy, bias=MAGIC, scale=float(inv_s0))
                r1 = pt.tile([128, w_], F32, tag="r1")
                nc.scalar.activation(r1, wr, AF.Copy, bias=MAGIC, scale=float(inv_s1))
                c0 = pt.tile([128, w_], F32, tag="c0")
                p2 = pt.tile([128, w_], F32, tag="p2")
                q1t = pt.tile([128, w_], F32, tag="q1t")
                nc.vector.tensor_scalar(c0, r0, MAGIC - 8.0, MAGIC + 7.0, AL.max, AL.min)
                nc.gpsimd.tensor_scalar(
                    p2, c0, -MAGIC, float(q0[R][Cc]), AL.add, AL.mult
                )
                nc.gpsimd.tensor_scalar(
                    q1t, r1, -MAGIC, float(q1[R][Cc]), AL.add, AL.mult
                )
                o0 = i * O_SPAN + c_lo
                nc.vector.tensor_tensor(
                    out=we_tiles[sp][:, o0 : o0 + w_], in0=p2, in1=q1t, op=AL.add
                )

            out_r = out_t[:, :].rearrange("(g p) t -> p g t", p=128)

            def mm_group(sp, tcn, ot, ob):
                """8 K-tile matmuls into one PSUM bank + drain into ob."""
                we = we_tiles[sp]
                ps = psum.tile([128, TCH], F32, tag="ps")
                for i in range(NI):
                    nc.tensor.matmul(
                        ps,
                        we[:, i * O_SPAN + 128 * ot : i * O_SPAN + 128 * (ot + 1)],
                        xq_all[:, i * T_CORE + TCH * tcn : i * T_CORE + TCH * (tcn + 1)],
                        start=(i == 0),
                        stop=(i == NI - 1),
                    )
                og = sp * NOT_ + ot
                obsl = ob[:, ot * TCH : (ot + 1) * TCH]
                if ot % 2 == 0:
                    nc.scalar.activation(
                        obsl, ps, AF.Identity, bias=b_t[:, og : og + 1], scale=1.0
                    )
                else:
                    nc.vector.tensor_scalar(
                        obsl, ps, 1.0, b_t[:, og : og + 1], AL.mult, AL.add
                    )

            gctr = [0]

            def mm_block(sp, tcn, bg=None, xg=None):
                """One (span, t-chunk): 4 o-tile groups; out DMA per 2 groups.
                bg: next spans' quant chains, paced 1/group early then 1 per
                2 groups (so chain work never swamps the drain queues). xg:
                next x-chunk loads, 2 per group."""
                ob = pob.tile([128, NOT_ * TCH], BF16, tag="ob")
                for ot in range(NOT_):
                    mm_group(sp, tcn, ot, ob)
                    g = gctr[0]
                    gctr[0] += 1
                    # 8 chains per 16-group span window, front-loaded: the
                    # next span's W is ready ~5 groups before it is needed.
                    lg = g % 16
                    if bg is not None and (lg < 4 or (lg < 12 and lg % 2 == 0)):
                        next(bg, None)
                    if xg is not None:
                        next(xg, None)
                        next(xg, None)
                    if ot % 2 == 1:
                        h = ot // 2
                        src = ob[:, h * 2 * TCH : (h + 1) * 2 * TCH].rearrange(
                            "p (o t) -> p o t", o=2
                        )
                        g0 = sp * NOT_ + h * 2
                        dst = out_r[:, g0 : g0 + 2, TCH * tcn : TCH * (tcn + 1)]
                        out_q.append((dst, src))
                        while len(out_q) > 4:
                            d, s_ = out_q.pop(0)
                            nc.sync.dma_start(out=d, in_=s_)

            def quant_span_iter(sp):
                """Overlapped spans: one batched 3D W DMA, then 8 chains."""
                wb = pwb.tile([128, NI * O_SPAN], F32, tag="wb")
                nc.sync.dma_start(
                    out=wb[:, :].rearrange("p (i o) -> p i o", i=NI),
                    in_=w_r[:, :, sp * O_SPAN : (sp + 1) * O_SPAN],
                )
                for i in range(NI):
                    quant_chain(sp, i, wr=wb[:, i * O_SPAN : (i + 1) * O_SPAN])
                    yield

            # ---- emission schedule ----
            # span-0 W first (half-width chains for latency), x tc0 after the
            # W DMAs so the PE's first accumulation group is ready early.
            for i in range(NI):
                quant_chain(0, i, 0, O_SPAN // 2)
            for _ in load_x_chunk(0, nsplit=2):
                pass
            for i in range(NI):
                quant_chain(0, i, O_SPAN // 2, O_SPAN)

            def bg_seq():
                for sp in range(1, NSP):
                    yield from quant_span_iter(sp)

            def xg_seq():
                for tcn in range(1, NTC):
                    yield from load_x_chunk(tcn)

            out_q = []
            bg = bg_seq()
            xg = xg_seq()
            for sp in range(NSP):
                for tcn in range(NTC):
                    mm_block(sp, tcn, bg, xg)
            for _ in bg:
                pass
            for _ in xg:
                pass
            for d, s_ in out_q:
                nc.sync.dma_start(out=d, in_=s_)
    nc.compile()
    return nc


def _derive(arch_weights, w_scales):
    aw = np.asarray(arch_weights, dtype=np.float64)
    S4 = aw.reshape(2, 2, 2, 2)  # [h_idx, it_idx, m, n]
    C = float(aw.sum())
    s0 = float(np.asarray(w_scales)[0])  # 4-bit scale
    s1 = float(np.asarray(w_scales)[1])  # 8-bit scale
    Ssum = S4.sum(axis=2)  # [h, it, n]
    G = np.zeros((2, 2, 2))  # [n, R, Cc]
    for n in (0, 1):
        for R in (0, 1):
            its = (0, 1) if R == 0 else (1,)
            for Cc in (0, 1):
                hs = (0, 1) if Cc == 0 else (1,)
                G[n, R, Cc] = sum(Ssum[h, it, n] for it in its for h in hs)
    q0 = (C * G[0] * s0).astype(np.float32)  # [R][Cc]
    q1 = (C * G[1] * s1).astype(np.float32)
    beta0 = np.float64(C)
    beta1 = np.float64(S4[:, 1].sum())
    inv_s0 = np.float32(1.0 / s0)
    inv_s1 = np.float32(1.0 / s1)
    return inv_s0, inv_s1, q0, q1, beta0, beta1, s0, s1


def _fallback(x, arch_weights, weight, bias, a_scales, w_scales):
    """Exact numpy replica of the reference (guard path; not used for the
    shipped input distribution)."""
    aw = np.asarray(arch_weights, np.float32)
    x = np.asarray(x, np.float32)
    w = np.asarray(weight, np.float32)
    b = np.asarray(bias, np.float32)
    a_s = np.asarray(a_scales, np.float32)
    w_s = np.asarray(w_scales, np.float32)
    rows = np.arange(O_DIM)[:, None]
    cols = np.arange(I_DIM)[None, :]

    def fq(v, scale, bit):
        qn, qp = -(2.0 ** (bit - 1)), 2.0 ** (bit - 1) - 1
        return (np.round(np.clip(v / scale, qn, qp)) * scale).astype(np.float32)

    x_mix = np.zeros_like(x)
    w_mix = np.zeros_like(w)
    b_mix = np.zeros_like(b)
    k = 0
    for h in (768, 1024):
        for it in (3072, 4096):
            mask = ((rows < it) & (cols < h)).astype(np.float32)
            w_pad = w * mask
            b_pad = b * (rows[:, 0] < it).astype(np.float32)
            for m, ab in enumerate((4, 8)):
                for n, wb in enumerate((4, 8)):
                    wk = aw[k]
                    x_mix = x_mix + wk * fq(x, a_s[m], ab)
                    w_mix = w_mix + wk * fq(w_pad, w_s[n], wb)
                    b_mix = b_mix + wk * b_pad
                    k += 1
    return (
        np.einsum("bsi,oi->bso", x_mix, w_mix, optimize=True) + b_mix
    ).astype(np.float32)


def _o_blocks(jo):
    """Interleaved 512-row o-blocks owned by core half jo (R=1 only last)."""
    return [jo + 2 * k for k in range(NSP)]


def _run(inputs, trace=False):
    x = np.ascontiguousarray(np.asarray(inputs["x"], np.float32))
    arch_weights = np.asarray(inputs["arch_weights"], np.float32)
    weight = np.ascontiguousarray(np.asarray(inputs["weight"], np.float32))
    bias = np.ascontiguousarray(np.asarray(inputs["bias"], np.float32))
    a_scales = np.asarray(inputs["a_scales"], np.float32)
    w_scales = np.asarray(inputs["w_scales"], np.float32)

    inv_s0, inv_s1, q0, q1, beta0, beta1, s0, s1 = _derive(arch_weights, w_scales)

    # fast-path validity (always true for the shipped input distribution)
    if not (
        np.all(np.abs(a_scales - 1.0) == 0.0)
        and float(np.abs(x).max()) < 7.49
        and float(np.abs(weight).max()) / s1 < 126.9
    ):
        return _fallback(x, arch_weights, weight, bias, a_scales, w_scales), None

    key = (
        float(inv_s0), float(inv_s1), tuple(np.asarray(q0).ravel().tolist()),
        tuple(np.asarray(q1).ravel().tolist()),
    )
    if key not in _cache:
        _cache.clear()
        _cache[key] = _build(inv_s0, inv_s1, q0, q1)
    nc = _cache[key]

    x2 = x.reshape(T_TOT, I_DIM)
    beta = np.where(np.arange(O_DIM) < 3072, beta0, beta1)
    b_fold = (bias.astype(np.float64) * beta).astype(np.float32)

    in_maps = []
    for j in range(N_CORES):
        jt, jo = j % NJT, j // NJT
        x_sh = np.ascontiguousarray(x2[jt * T_CORE : (jt + 1) * T_CORE].T)
        blocks = _o_blocks(jo)
        osel = np.concatenate(
            [np.arange(bk * O_SPAN, (bk + 1) * O_SPAN) for bk in blocks]
        )
        w_sh = np.ascontiguousarray(weight[osel].T)  # [I_DIM, O_CORE]
        b_sh = np.ascontiguousarray(
            b_fold[osel].reshape(O_CORE // 128, 128).T
        )
        in_maps.append({"x_t": x_sh, "w_t": w_sh, "b_pt": b_sh})

    res = bass_utils.run_bass_kernel_spmd(
        nc, in_maps, core_ids=list(range(N_CORES)), trace=trace
    )
    global _last_res
    _last_res = res
    out = np.empty((T_TOT, O_DIM), np.float32)
    for j in range(N_CORES):
        jt, jo = j % NJT, j // NJT
        o_t = res.results[j]["out_t"].astype(np.float32)  # [O_CORE, T_CORE]
        for k, bk in enumerate(_o_blocks(jo)):
            out[
                jt * T_CORE : (jt + 1) * T_CORE,
                bk * O_SPAN : (bk + 1) * O_SPAN,
            ] = o_t[k * O_SPAN : (k + 1) * O_SPAN].T
    return out.reshape(B, S, O_DIM), res.exec_time_ns


def kernel(**inputs):
    out, _ = _run(inputs, trace=False)
    return out


# revision 61
# speedup vs baseline: 1.0106x; 1.0106x over previous
"""Trainium2 Bass kernel for nn_MixedLinear (DARTS-style mixed-precision supernet linear).

Reference math (16-term arch-weighted mixture) reduces algebraically to:

  x_mix = C * round(x)                      C = sum(arch_weights)
  w_mix[o,i] = G0(R,Cc)*s0*clip(round(w/s0),-8,7) + G1(R,Cc)*s1*round(w/s1)
  out = x_mix @ w_mix^T + beta(R) * bias
      = round(x) @ W_eff^T + b_mix,   W_eff = C * w_mix

with region grid R = (o >= 3072), Cc = (i >= 768); see the fallback for the
unreduced form.

v4 design (vs the fp32r data-parallel v1 at ~170us):
  - Hybrid sharding: tokens 4-way x output-rows 2-way, with the o-rows
    assigned in INTERLEAVED 512-row blocks (core half jo takes blocks
    jo, jo+2, jo+4, jo+6). Both halves then see the same R-pattern
    (R=1 only in their last block), so a single SPMD program with the
    (R,Cc) dequant scales baked as immediates serves all 8 cores.
    Halves the per-core W-dequant elementwise work and HBM vs pure DP.
  - bf16 stationary (W_eff) and moving (round(x), exact in bf16): the PE
    streams 1 row/cycle either way but LDWEIGHTS halves, and W_eff
    rounding to bf16 costs only ~1e-3 relative output error.
    (fp8+DoubleRow was tried: DR matmuls measure 216ns vs bf16 259ns per
    512-row instr, but the A+B residual split doubles the FLOPs and
    fp8 elementwise on DVE/Pool is 4-8x slower than fp32 -- net loss.)
  - Output written bf16 (halves output HBM); host upcasts.
  - Elementwise spread across Act/DVE/Pool (Pool gets float-immediate
    tensor_scalars only); PSUM drains alternate Act/DVE and apply the
    beta-folded host-prepared bias in the same op.
  - First span's chains are balanced for latency so the PE starts ~10us
    in; later spans' chains lean on Pool (which idles during matmuls).

Rounding on device uses the magic-number trick: fp32 (v + 1.5*2^23) -
1.5*2^23 == round-half-even(v), matching jnp.round exactly.
"""

import numpy as np

import concourse.mybir as mybir
from concourse import bacc, bass_utils
from concourse.tile import TileContext

N_CORES = 8
NJT, NJO = 4, 2          # token shards x output shards
B, S, I_DIM, O_DIM = 4, 2048, 1024, 4096
T_TOT = B * S
T_CORE = T_TOT // NJT    # 2048 tokens per core
O_CORE = O_DIM // NJO    # 2048 output rows per core
NI = I_DIM // 128        # 8 contraction tiles
NSP = 4                  # o-spans (interleaved 512-blocks) per core
O_SPAN = O_CORE // NSP   # 512
NOT_ = O_SPAN // 128     # 4 o-tiles per span
TCH = 512                # matmul moving free dim
NTC = T_CORE // TCH      # 4 t-chunks
MAGIC = 12582912.0       # 1.5 * 2**23
F32 = mybir.dt.float32
BF16 = mybir.dt.bfloat16
AL = mybir.AluOpType
AF = mybir.ActivationFunctionType

_cache: dict = {}
_last_res = None


def _build(inv_s0, inv_s1, q0, q1):
    """Build + compile the per-core kernel. q0/q1 are 2x2 (R, Cc) grids,
    baked as immediates; span sp uses R = (sp == NSP-1) on every core."""
    nc = bacc.Bacc("TRN2", target_bir_lowering=False)
    x_t = nc.dram_tensor("x_t", [I_DIM, T_CORE], F32, kind="ExternalInput")
    w_t = nc.dram_tensor("w_t", [I_DIM, O_CORE], F32, kind="ExternalInput")
    b_pt = nc.dram_tensor("b_pt", [128, O_CORE // 128], F32, kind="ExternalInput")
    out_t = nc.dram_tensor("out_t", [O_CORE, T_CORE], BF16, kind="ExternalOutput")

    with TileContext(nc) as tc:
        with (
            tc.tile_pool(name="pper", bufs=1) as pper,
            tc.tile_pool(name="pxs", bufs=3) as pxs,
            tc.tile_pool(name="pws", bufs=4) as pws,
            tc.tile_pool(name="pwb", bufs=2) as pwb,
            tc.tile_pool(name="pt", bufs=3) as pt,
            tc.tile_pool(name="pob", bufs=5) as pob,
            tc.tile_pool(name="psum", bufs=8, space="PSUM") as psum,
        ):
            b_t = pper.tile([128, O_CORE // 128], F32, tag="bt")
            nc.sync.dma_start(out=b_t, in_=b_pt[:, :])

            # xq = round(x^T) in bf16 (exact: ints in [-7,7]). Layout [p, i*T+t].
            xq_all = pper.tile([128, NI * T_CORE], BF16, tag="xq")
            x_r = x_t[:, :].rearrange("(i p) t -> p i t", p=128)
            w_r = w_t[:, :].rearrange("(i p) o -> p i o", p=128)

            def load_x_chunk(tcn, nsplit=1):
                """Load one t-chunk for all 8 k-tiles with nsplit batched 3D
                DMAs (one SP issue covers NI/nsplit tiles), then round."""
                step = NI // nsplit
                xb = pxs.tile([128, NI * TCH], F32, tag="xb")
                for s in range(nsplit):
                    nc.sync.dma_start(
                        out=xb[:, s * step * TCH : (s + 1) * step * TCH].rearrange(
                            "p (i t) -> p i t", i=step
                        ),
                        in_=x_r[:, s * step : (s + 1) * step, TCH * tcn : TCH * (tcn + 1)],
                    )
                for i in range(NI):
                    o0 = i * T_CORE + TCH * tcn
                    # dtype-converting ops are fast only on DVE/Act (Pool's
                    # software path is ~12x slower for bf16 writes / max-min)
                    nc.vector.tensor_scalar(
                        xq_all[:, o0 : o0 + TCH],
                        xb[:, i * TCH : (i + 1) * TCH],
                        MAGIC, MAGIC, AL.add, AL.subtract,
                    )
                    yield

            # W_eff stationary bf16 per span, layout [p, i*O_SPAN + o]
            we_tiles = [
                pper.tile([128, NI * O_SPAN], BF16, tag=f"we{sp}", name=f"we{sp}")
                for sp in range(NSP)
            ]

            def quant_chain(sp, i, c_lo=0, c_hi=O_SPAN, wr=None):
                """Dequant chain for (sp, i), span-columns [c_lo, c_hi).
                max/min never goes to Pool (12x slow there); Pool gets only
                add/mult; Act gets the affine rounds; DVE clip + combine."""
                R = 1 if sp == NSP - 1 else 0
                Cc = 1 if i * 128 >= 768 else 0
                w_ = c_hi - c_lo
                if wr is None:
                    wr = pws.tile([128, w_], F32, tag="wr")
                    nc.sync.dma_start(
                        out=wr,
                        in_=w_t[
                            128 * i : 128 * (i + 1),
                            sp * O_SPAN + c_lo : sp * O_SPAN + c_hi,
                        ],
                    )
                r0 = pt.tile([128, w_], F32, tag="r0")
                nc.scalar.activation(r0, wr, AF.Copy, bias=MAGIC, scale=float(inv_s0))
                r1 = pt.tile([128, w_], F32, tag="r1")
                nc.scalar.activation(r1, wr, AF.Copy, bias=MAGIC, scale=float(inv_s1))
                c0 = pt.tile([128, w_], F32, tag="c0")
                p2 = pt.tile([128, w_], F32, tag="p2")
                q1t = pt.tile([128, w_], F32, tag="q1t")
                nc.vector.tensor_scalar(c0, r0, MAGIC - 8.0, MAGIC + 7.0, AL.max, AL.min)
                nc.gpsimd.tensor_scalar(
                    p2, c0, -MAGIC, float(q0[R][Cc]), AL.add, AL.mult
                )
                nc.gpsimd.tensor_scalar(
                    q1t, r1, -MAGIC, float(q1[R][Cc]), AL.add, AL.mult
                )
                o0 = i * O_SPAN + c_lo
                nc.vector.tensor_tensor(
                    out=we_tiles[sp][:, o0 : o0 + w_], in0=p2, in1=q1t, op=AL.add
                )

            out_r = out_t[:, :].rearrange("(g p) t -> p g t", p=128)

            def mm_group(sp, tcn, ot, ob):
                """8 K-tile matmuls into one PSUM bank + drain into ob."""
                we = we_tiles[sp]
                ps = psum.tile([128, TCH], F32, tag="ps")
                for i in range(NI):
                    nc.tensor.matmul(
                        ps,
                        we[:, i * O_SPAN + 128 * ot : i * O_SPAN + 128 * (ot + 1)],
                        xq_all[:, i * T_CORE + TCH * tcn : i * T_CORE + TCH * (tcn + 1)],
                        start=(i == 0),
                        stop=(i == NI - 1),
                    )
                og = sp * NOT_ + ot
                obsl = ob[:, ot * TCH : (ot + 1) * TCH]
                if ot % 2 == 0:
                    nc.scalar.activation(
                        obsl, ps, AF.Identity, bias=b_t[:, og : og + 1], scale=1.0
                    )
                else:
                    nc.vector.tensor_scalar(
                        obsl, ps, 1.0, b_t[:, og : og + 1], AL.mult, AL.add
                    )

            gctr = [0]

            def mm_block(sp, tcn, bg=None, xg=None):
                """One (span, t-chunk): 4 o-tile groups; out DMA per 2 groups.
                bg: next spans' quant chains, paced 1/group early then 1 per
                2 groups (so chain work never swamps the drain queues). xg:
                next x-chunk loads, 2 per group."""
                ob = pob.tile([128, NOT_ * TCH], BF16, tag="ob")
                for ot in range(NOT_):
                    mm_group(sp, tcn, ot, ob)
                    g = gctr[0]
                    gctr[0] += 1
                    # 8 chains per 16-group span window, front-loaded: the
                    # next span's W is ready ~5 groups before it is needed.
                    lg = g % 16
                    if bg is not None and (lg < 4 or (lg < 12 and lg % 2 == 0)):
                        next(bg, None)
                    if xg is not None:
                        next(xg, None)
                        next(xg, None)
                    if ot == NOT_ - 1:
                        # one fused out-DMA per block (SP issue cost is flat
                        # ~640ns per DMA regardless of size)
                        src = ob[:, :].rearrange("p (o t) -> p o t", o=NOT_)
                        g0 = sp * NOT_
                        dst = out_r[:, g0 : g0 + NOT_, TCH * tcn : TCH * (tcn + 1)]
                        out_q.append((dst, src))
                        while len(out_q) > 2:
                            d, s_ = out_q.pop(0)
                            nc.sync.dma_start(out=d, in_=s_)

            def quant_span_iter(sp):
                """Overlapped spans: one batched 3D W DMA, then 8 chains."""
                wb = pwb.tile([128, NI * O_SPAN], F32, tag="wb")
                nc.sync.dma_start(
                    out=wb[:, :].rearrange("p (i o) -> p i o", i=NI),
                    in_=w_r[:, :, sp * O_SPAN : (sp + 1) * O_SPAN],
                )
                for i in range(NI):
                    quant_chain(sp, i, wr=wb[:, i * O_SPAN : (i + 1) * O_SPAN])
                    yield

            # ---- emission schedule ----
            # span-0 W first (half-width chains for latency), x tc0 after the
            # W DMAs so the PE's first accumulation group is ready early.
            for i in range(NI):
                quant_chain(0, i, 0, O_SPAN // 2)
            for _ in load_x_chunk(0, nsplit=2):
                pass
            for i in range(NI):
                quant_chain(0, i, O_SPAN // 2, O_SPAN)

            def bg_seq():
                for sp in range(1, NSP):
                    yield from quant_span_iter(sp)

            def xg_seq():
                for tcn in range(1, NTC):
                    yield from load_x_chunk(tcn)

            out_q = []
            bg = bg_seq()
            xg = xg_seq()
            for sp in range(NSP):
                for tcn in range(NTC):
                    mm_block(sp, tcn, bg, xg)
            for _ in bg:
                pass
            for _ in xg:
                pass
            for d, s_ in out_q:
                nc.sync.dma_start(out=d, in_=s_)
    nc.compile()
    return nc


def _derive(arch_weights, w_scales):
    aw = np.asarray(arch_weights, dtype=np.float64)
    S4 = aw.reshape(2, 2, 2, 2)  # [h_idx, it_idx, m, n]
    C = float(aw.sum())
    s0 = float(np.asarray(w_scales)[0])  # 4-bit scale
    s1 = float(np.asarray(w_scales)[1])  # 8-bit scale
    Ssum = S4.sum(axis=2)  # [h, it, n]
    G = np.zeros((2, 2, 2))  # [n, R, Cc]
    for n in (0, 1):
        for R in (0, 1):
            its = (0, 1) if R == 0 else (1,)
            for Cc in (0, 1):
                hs = (0, 1) if Cc == 0 else (1,)
                G[n, R, Cc] = sum(Ssum[h, it, n] for it in its for h in hs)
    q0 = (C * G[0] * s0).astype(np.float32)  # [R][Cc]
    q1 = (C * G[1] * s1).astype(np.float32)
    beta0 = np.float64(C)
    beta1 = np.float64(S4[:, 1].sum())
    inv_s0 = np.float32(1.0 / s0)
    inv_s1 = np.float32(1.0 / s1)
    return inv_s0, inv_s1, q0, q1, beta0, beta1, s0, s1


def _fallback(x, arch_weights, weight, bias, a_scales, w_scales):
    """Exact numpy replica of the reference (guard path; not used for the
    shipped input distribution)."""
    aw = np.asarray(arch_weights, np.float32)
    x = np.asarray(x, np.float32)
    w = np.asarray(weight, np.float32)
    b = np.asarray(bias, np.float32)
    a_s = np.asarray(a_scales, np.float32)
    w_s = np.asarray(w_scales, np.float32)
    rows = np.arange(O_DIM)[:, None]
    cols = np.arange(I_DIM)[None, :]

    def fq(v, scale, bit):
        qn, qp = -(2.0 ** (bit - 1)), 2.0 ** (bit - 1) - 1
        return (np.round(np.clip(v / scale, qn, qp)) * scale).astype(np.float32)

    x_mix = np.zeros_like(x)
    w_mix = np.zeros_like(w)
    b_mix = np.zeros_like(b)
    k = 0
    for h in (768, 1024):
        for it in (3072, 4096):
            mask = ((rows < it) & (cols < h)).astype(np.float32)
            w_pad = w * mask
            b_pad = b * (rows[:, 0] < it).astype(np.float32)
            for m, ab in enumerate((4, 8)):
                for n, wb in enumerate((4, 8)):
                    wk = aw[k]
                    x_mix = x_mix + wk * fq(x, a_s[m], ab)
                    w_mix = w_mix + wk * fq(w_pad, w_s[n], wb)
                    b_mix = b_mix + wk * b_pad
                    k += 1
    return (
        np.einsum("bsi,oi->bso", x_mix, w_mix, optimize=True) + b_mix
    ).astype(np.float32)


def _o_blocks(jo):
    """Interleaved 512-row o-blocks owned by core half jo (R=1 only last)."""
    return [jo + 2 * k for k in range(NSP)]


def _run(inputs, trace=False):
    x = np.ascontiguousarray(np.asarray(inputs["x"], np.float32))
    arch_weights = np.asarray(inputs["arch_weights"], np.float32)
    weight = np.ascontiguousarray(np.asarray(inputs["weight"], np.float32))
    bias = np.ascontiguousarray(np.asarray(inputs["bias"], np.float32))
    a_scales = np.asarray(inputs["a_scales"], np.float32)
    w_scales = np.asarray(inputs["w_scales"], np.float32)

    inv_s0, inv_s1, q0, q1, beta0, beta1, s0, s1 = _derive(arch_weights, w_scales)

    # fast-path validity (always true for the shipped input distribution)
    if not (
        np.all(np.abs(a_scales - 1.0) == 0.0)
        and float(np.abs(x).max()) < 7.49
        and float(np.abs(weight).max()) / s1 < 126.9
    ):
        return _fallback(x, arch_weights, weight, bias, a_scales, w_scales), None

    key = (
        float(inv_s0), float(inv_s1), tuple(np.asarray(q0).ravel().tolist()),
        tuple(np.asarray(q1).ravel().tolist()),
    )
    if key not in _cache:
        _cache.clear()
        _cache[key] = _build(inv_s0, inv_s1, q0, q1)
    nc = _cache[key]

    x2 = x.reshape(T_TOT, I_DIM)
    beta = np.where(np.arange(O_DIM) < 3072, beta0, beta1)
    b_fold = (bias.astype(np.float64) * beta).astype(np.float32)

    in_maps = []
    for j in range(N_CORES):
        jt, jo = j % NJT, j // NJT
        x_sh = np.ascontiguousarray(x2[jt * T_CORE : (jt + 1) * T_CORE].T)
        blocks = _o_blocks(jo)
        osel = np.concatenate(
            [np.arange(bk * O_SPAN, (bk + 1) * O_SPAN) for bk in blocks]
        )
        w_sh = np.ascontiguousarray(weight[osel].T)  # [I_DIM, O_CORE]
        b_sh = np.ascontiguousarray(
            b_fold[osel].reshape(O_CORE // 128, 128).T
        )
        in_maps.append({"x_t": x_sh, "w_t": w_sh, "b_pt": b_sh})

    res = bass_utils.run_bass_kernel_spmd(
        nc, in_maps, core_ids=list(range(N_CORES)), trace=trace
    )
    global _last_res
    _last_res = res
    out = np.empty((T_TOT, O_DIM), np.float32)
    for j in range(N_CORES):
        jt, jo = j % NJT, j // NJT
        o_t = res.results[j]["out_t"].astype(np.float32)  # [O_CORE, T_CORE]
        for k, bk in enumerate(_o_blocks(jo)):
            out[
                jt * T_CORE : (jt + 1) * T_CORE,
                bk * O_SPAN : (bk + 1) * O_SPAN,
            ] = o_t[k * O_SPAN : (k + 1) * O_SPAN].T
    return out.reshape(B, S, O_DIM), res.exec_time_ns


def kernel(**inputs):
    out, _ = _run(inputs, trace=False)
    return out
